# revision 1
# baseline (speedup 1.0000x reference)
"""Trainium2 Bass kernel for nn_LinearEncoder (gnn_message_passing).

Reference computes, for N=512 nodes with n_in = n_out = 256:
    i, j = triu_indices(N, k=1)
    edges = concat([x[i], x[j]], -1)            # [E, 512]
    h = edges @ W.T + b                         # [E, n_out]
    out[i, j] = h ; out = out + out.T           # [N, N, 256], 0 diagonal

Key algebraic identity: with W = [W1 | W2],
    h(i, j) = A[i] + B[j] + b,   A = x @ W1.T,  B = x @ W2.T
so the full output is
    out[i, j] = A[min(i,j)] + B'[max(i,j)]      (B' = B + b), 0 on diagonal.

Sharding: output rows split across 8 cores (64 rows each), one SPMD
program.  Core k receives x pre-rotated by its row base
(x_rot[t] = x[(base+t) % 512]) so the triangular "diagonal block" sits at
local columns s in [0, 64) on every core; region selection (A vs B')
enters only through small 0/1 mask *inputs*.

End-to-end bf16 datapath (host pre-casts inputs, bf16 output slabs
re-cast to f32 on host — well inside the tolerance), halving the HBM
write roofline (~17 MB/core).  Per row-pair rp (rows r0 = 2rp, r0+1):
  - five concurrent small-K masked-broadcast matmuls in distinct PE
    row-groups produce the row terms of the three main 128-wide column
    blocks and the two block0-upper halves; the only full-array matmul
    is the combined L+R constant selection for the triangular diagonal
    block (exact-zero diagonal);
  - evacuation is spread over every engine: VectorE adds the bf16
    column tables into J1|J2 (one 2-bank op), ScalarE copies the raw J3
    row term and [diag|upper], and the remaining column tables are
    folded by cheap in-place bf16 SBUF adds (DVE 2x mode / GpSimd);
  - per group of 4 row-pairs everything lands in one [128, 8192] bf16
    tile shipped as two 1 MB HWDGE DMAs (16 total per core), making the
    HBM write stream the sole bottleneck.
Preamble latency is managed explicitly: the uint8 diag selectors cast
to bf16 during the DMA, the flat row tables (the loop's gating
dependency, inherently 512B-descriptor transfers) are split into a fast
16-node chunk covering the first 8 row-pairs plus the remainder, and
all eight flattens issue together across both HWDGE queues.
"""

import contextlib
import os
import sys

for _p in ("/opt/trn_rl_repo", "/root/.axon_site/_ro/trn_rl_repo"):
    if os.path.isdir(_p) and _p not in sys.path:
        sys.path.insert(0, _p)

import numpy as np
import ml_dtypes

import concourse.bass as bass
import concourse.bacc as bacc
import concourse.mybir as mybir
import concourse.tile as tile
from concourse.bass_utils import run_bass_kernel_spmd

N = 512
CH = 256          # n_out
NIN = 256         # n_in
NCORES = 8
RB = N // NCORES  # 64 rows per core
F32 = mybir.dt.float32
BF16 = mybir.dt.bfloat16
BF16NP = ml_dtypes.bfloat16


# --------------------------------------------------------------------------
# host-side constant builders
# --------------------------------------------------------------------------

def _masks_RL(k: int):
    """R/L region indicators over local columns s for core k."""
    base = RB * k
    wrap = N - base  # columns s >= wrap hold wrapped (j < base) entries
    s = np.arange(N)
    R = ((s >= 64) & (s < wrap)).astype(np.float32)
    L = (s >= wrap).astype(np.float32)
    return R, L


def _diag_const():
    """Combined masked-selection weights for the 64x64 diagonal blocks.

    For row-pair rp, output column m = q*64 + s (q in {0,1}, s in [0,64)),
    with r_q = 2*rp + q and rhs dcb = [A_rot[0:64] ; B'_rot[0:64]] (K = 128):
      s < r_q:  value = A_rot[s] + B'_rot[r_q]
      s > r_q:  value = B'_rot[s] + A_rot[r_q]
      s == r_q: all weights zero -> exact 0 output.
    """
    d = np.zeros((128, 32 * 128), np.float32)
    for rp in range(32):
        for q in range(2):
            r_q = 2 * rp + q
            for s in range(64):
                m = rp * 128 + q * 64 + s
                if s < r_q:
                    d[s, m] = 1.0            # A_rot[s]
                    d[64 + r_q, m] = 1.0     # B'_rot[r_q]
                elif s > r_q:
                    d[64 + s, m] = 1.0       # B'_rot[s]
                    d[r_q, m] = 1.0          # A_rot[r_q]
    return d


def _shared_inputs(W: np.ndarray, b: np.ndarray):
    W = np.asarray(W, np.float32)
    b = np.asarray(b, np.float32)
    w12 = np.concatenate(
        [np.ascontiguousarray(W[:, :NIN].T), np.ascontiguousarray(W[:, NIN:].T)],
        axis=1)                                     # [in, 2*out] = [A | B]
    b2bc = np.broadcast_to(
        np.concatenate([np.zeros(CH, np.float32), b]), (128, 2 * CH))
    return {
        "w12t": w12.astype(BF16NP),
        "b2bc": np.ascontiguousarray(b2bc).astype(BF16NP),
        "dcomb": _diag_const().astype(np.uint8),
    }


def _core_inputs(x: np.ndarray, k: int):
    x = np.asarray(x, np.float32)
    base = RB * k
    x_rot = np.roll(x, -base, axis=0)
    R, L = _masks_RL(k)

    # column-table masks: cm[:, s] = R over node block s, cm[:, 4+s] = L
    cm = np.zeros((128, 8), np.float32)
    for t in range(4):
        cm[:, t] = R[128 * t:128 * (t + 1)]
        cm[:, 4 + t] = L[128 * t:128 * (t + 1)]

    # wmt [128, 512]: per-row-group small-K masked-broadcast weights.
    #   rows 0-1   cols 128:256 -> J=1   (w0 = R, w1 = L over that block)
    #   rows 32-33 cols 256:384 -> J=2
    #   rows 64-65 cols 384:512 -> J=3
    #   rows 96-97 cols 0:128   -> block0-upper row r0 (p<64: s=64+p),
    #     cols 128:256 -> row r0+1 (p>=64: s=p), rhs offset +256
    wm = np.zeros((128, 512), np.float32)
    for J in (1, 2, 3):
        gp = 32 * (J - 1)
        wm[gp, 128 * J:128 * (J + 1)] = R[128 * J:128 * (J + 1)]
        wm[gp + 1, 128 * J:128 * (J + 1)] = L[128 * J:128 * (J + 1)]
    p = np.arange(64)
    wm[96, p] = R[64 + p]
    wm[97, p] = L[64 + p]
    wm[96, 128 + 64 + p] = R[64 + p]
    wm[97, 128 + 64 + p] = L[64 + p]
    return {
        "xt_rot": np.ascontiguousarray(x_rot.T).astype(BF16NP),  # [in, node]
        "cm": cm,
        "wm": wm.astype(BF16NP),
    }


# --------------------------------------------------------------------------
# device program
# --------------------------------------------------------------------------

_PROGRAM = None


def _build_program() -> bass.Bass:
    nc = bacc.Bacc()
    f32 = F32

    # ---- dram tensors -----------------------------------------------------
    xt_rot = nc.dram_tensor("xt_rot", [NIN, N], BF16, kind="ExternalInput")
    w12t = nc.dram_tensor("w12t", [NIN, 2 * CH], BF16, kind="ExternalInput")
    b2bc = nc.dram_tensor("b2bc", [128, 2 * CH], BF16, kind="ExternalInput")
    cm = nc.dram_tensor("cm", [128, 8], F32, kind="ExternalInput")
    d_wm = nc.dram_tensor("wm", [128, N], BF16, kind="ExternalInput")
    d_dc = nc.dram_tensor("dcomb", [128, 32 * 128], mybir.dt.uint8,
                          kind="ExternalInput")

    # slab[g, p, :]: two 4096-col halves h = sub//2 (u = sub%2), each the
    # DMA unit: base = 4096h; J12 at base+1024u (512*Jh+256*q+c ->
    # row 8g+2sub+q, col 128(Jh+1)+p); J3 at base+2048+512u; [diag|upper]
    # at base+3072+512u (diag: p = q*64+s; upper: p<64: row r0, s=64+p ;
    # p>=64: row r0+1, s=p).  Host unpicks.
    slab = nc.dram_tensor("slab", [8, 128, 8192], BF16, kind="ExternalOutput")

    with tile.TileContext(nc) as tc:
        with (
            tc.tile_pool(name="const", bufs=1) as cpool,
            tc.tile_pool(name="tmp", bufs=2) as tpool,
            tc.tile_pool(name="psJ", bufs=2, space="PSUM") as psJ,
            tc.tile_pool(name="ps3", bufs=2, space="PSUM") as ps3,
            tc.tile_pool(name="psDU", bufs=2, space="PSUM") as psDU,
            tc.tile_pool(name="stS", bufs=2) as stS,
        ):
            # ---- load inputs (spread across HWDGE queues) ----------------
            def load(eng, dram, shape, dtype, tag):
                t = cpool.tile(shape, dtype, tag=tag)
                eng.dma_start(out=t[:], in_=dram[:])
                return t

            xt0 = load(nc.sync, xt_rot[0:128, :], [128, N], BF16, "xt0")
            w12a = load(nc.scalar, w12t[0:128, :], [128, 2 * CH], BF16, "w12a")
            # uint8 0/1 selectors cast to bf16 during the (SWDGE) DMA —
            # halves the largest input load
            dct = cpool.tile([128, 32 * 128], BF16, tag="dct")
            nc.gpsimd.dma_start(out=dct[:], in_=d_dc[:])
            xt1 = load(nc.sync, xt_rot[128:256, :], [128, N], BF16, "xt1")
            w12b = load(nc.scalar, w12t[128:256, :], [128, 2 * CH], BF16,
                        "w12b")
            b2t = load(nc.gpsimd, b2bc, [128, 2 * CH], BF16, "b2t")
            cmt = load(nc.gpsimd, cm, [128, 8], F32, "cmt")
            wmt = load(nc.gpsimd, d_wm, [128, N], BF16, "wmt")

            # ---- phase 1: tables [A | B'] (bf16), one psum per s-block ---
            # evac folds the (pre-broadcast) bias row in.  s=0 runs first,
            # entirely on DVE, so the flat-table DMAs (the loop's critical
            # dependency) can launch as early as possible.
            AB = [None] * 4
            for s in range(4):
                # phase-1 borrows the (wider) psJ pool so ps3 is free the
                # moment the main loop starts - J3(rp0) must be ready
                # early or the scheduler buries it behind J12 work.
                paw = psJ.tile([128, 1024], f32, tag="pj", name=f"ptb{s}")
                pa = paw[:, 0:2 * CH]
                mmd = nc.tensor.matmul
                mmd(pa, xt0[:, 128 * s:128 * (s + 1)], w12a[:],
                    start=True, stop=False)
                mmd(pa, xt1[:, 128 * s:128 * (s + 1)], w12b[:],
                    start=False, stop=True)
                comb = cpool.tile([128, 2 * CH], BF16, tag=f"AB{s}")
                if s == 0:
                    nc.vector.tensor_copy(out=comb[:, 0:CH],
                                          in_=paw[:, 0:CH])
                else:
                    nc.scalar.copy(out=comb[:, 0:CH], in_=paw[:, 0:CH])
                nc.vector.tensor_add(comb[:, CH:2 * CH], paw[:, CH:2 * CH],
                                     b2t[:, CH:2 * CH])
                AB[s] = comb
                if s == 0:
                    # flat row tables ft: row gp = A nodes 0..63 flattened,
                    # gp+1 = B'.  All eight direct flattens issue the
                    # moment AB[0] lands, balanced over the two HWDGE
                    # queues, so every row-group becomes ready together.
                    # chunked: nodes 0-15 first (covers rps 0-7, ~1/4 the
                    # descriptors, lands fast) so the loop starts early,
                    # then nodes 16-63 while the first groups process.
                    ft = cpool.tile([128, 64 * CH], BF16, tag="ft")
                    for lo, hi in ((0, 16), (16, 64)):
                        c0, c1 = lo * CH, hi * CH
                        for i, gp in enumerate((0, 32, 64, 96)):
                            qa = nc.sync if i % 2 == 0 else nc.scalar
                            qb = nc.scalar if i % 2 == 0 else nc.sync
                            qa.dma_start(out=ft[gp:gp + 1, c0:c1],
                                         in_=AB[0][lo:hi, 0:CH])
                            qb.dma_start(out=ft[gp + 1:gp + 2, c0:c1],
                                         in_=AB[0][lo:hi, CH:2 * CH])
                    # diag combined rhs dcb = [A_rot[0:64] ; B'_rot[0:64]]
                    dcb = cpool.tile([128, CH], BF16, tag="dcb")
                    nc.vector.tensor_copy(out=dcb[0:64, :],
                                          in_=AB[0][0:64, 0:CH])
                    nc.scalar.dma_start(out=dcb[64:128, :],
                                        in_=AB[0][0:64, CH:2 * CH])

            # ---- mixed column tables Cmix_s = R*B' + L*A (bf16) ----------
            Cmix = [None] * 4
            for s in range(4):
                t1 = tpool.tile([128, CH], BF16, tag="t1")
                nc.vector.tensor_scalar(t1[:], AB[s][:, 0:CH],
                                        cmt[:, 4 + s:5 + s], None,
                                        mybir.AluOpType.mult)
                cx = cpool.tile([128, CH], BF16, tag=f"C{s}")
                nc.vector.scalar_tensor_tensor(
                    cx[:], AB[s][:, CH:2 * CH], cmt[:, s:s + 1], t1[:],
                    mybir.AluOpType.mult, mybir.AluOpType.add)
                Cmix[s] = cx

            # duplicated column tables: CD3 = [C3|C3] (gates the first J3
            # evacuations - build first), CD12 = [C1|C1|C2|C2]
            CD12 = cpool.tile([128, 4 * CH], BF16, tag="CD12")
            CD3 = cpool.tile([128, 2 * CH], BF16, tag="CD3")
            for h in range(2):
                nc.scalar.copy(out=CD3[:, CH * h:CH * (h + 1)],
                               in_=Cmix[3][:])
            for h in range(2):
                nc.vector.tensor_copy(out=CD12[:, CH * h:CH * (h + 1)],
                                      in_=Cmix[1][:])
                nc.vector.tensor_copy(out=CD12[:, CH * (2 + h):CH * (3 + h)],
                                      in_=Cmix[2][:])
            # block0-upper column table, replicated to both q-halves
            cup = cpool.tile([128, CH], BF16, tag="cup")
            nc.scalar.dma_start(out=cup[0:64, :], in_=Cmix[0][64:128, :])
            nc.vector.tensor_copy(out=cup[64:128, :], in_=Cmix[0][64:128, :])

            # ---- phase 2: main loop --------------------------------------
            for g in range(8):
                S = stS.tile([128, 8192], BF16, tag="s", name=f"s_{g}")
                for sub in range(4):
                    rp = 4 * g + sub
                    off = 2 * rp * CH
                    base = 4096 * (sub // 2)
                    u = sub % 2
                    mm = nc.tensor.matmul
                    # the first shippable S-half gates the whole output
                    # stream: pin its work to the front of the scheduler's
                    # priority heap so it preempts run-ahead work the
                    # moment its dependencies clear.
                    hp = (tc.high_priority() if rp < 2
                          else contextlib.nullcontext())
                    ctx_stack = contextlib.ExitStack()
                    ctx_stack.enter_context(hp)
                    # four concurrent row-group matmuls (row terms)
                    pj = psJ.tile([128, 1024], f32, tag="pj",
                                  name=f"pj_{rp}")
                    mm(pj[:, 0:512], wmt[0:2, 128:256],
                       ft[0:2, off:off + 512],
                       start=True, stop=True, tile_position=(0, 0))
                    mm(pj[:, 512:1024], wmt[32:34, 256:384],
                       ft[32:34, off:off + 512],
                       start=True, stop=True, tile_position=(32, 0))
                    p3 = ps3.tile([128, 512], f32, tag="p3", name=f"p3_{rp}")
                    mm(p3[:], wmt[64:66, 384:512], ft[64:66, off:off + 512],
                       start=True, stop=True, tile_position=(64, 0))
                    pdu = psDU.tile([128, 512], f32, tag="pdu",
                                    name=f"pdu_{rp}")
                    mm(pdu[:, 256:512], wmt[96:98, 0:128],
                       ft[96:98, off:off + 256],
                       start=True, stop=False, tile_position=(96, 0))
                    mm(pdu[:, 256:512], wmt[96:98, 128:256],
                       ft[96:98, off + 256:off + 512],
                       start=False, stop=True, tile_position=(96, 0))
                    # the only full-array matmul: diag selection
                    mm(pdu[:, 0:256], dct[:, 128 * rp:128 * (rp + 1)],
                       dcb[:], start=True, stop=True)
                    # evacuation: ScalarE copies the raw J3 row term and
                    # [diag|upper-raw]; DVE adds CD12 into J1|J2; the
                    # column tables for J3/upper are folded by cheap
                    # in-place bf16 SBUF adds (2x mode) on DVE/GpSimd.
                    sl_j3 = S[:, base + 2048 + 512 * u:
                              base + 2048 + 512 * (u + 1)]
                    sl_du = S[:, base + 3072 + 512 * u:
                              base + 3072 + 512 * (u + 1)]
                    sl_up = S[:, base + 3072 + 512 * u + 256:
                              base + 3072 + 512 * (u + 1)]
                    nc.scalar.copy(out=sl_j3, in_=p3[:])
                    nc.scalar.copy(out=sl_du, in_=pdu[:])
                    nc.vector.tensor_add(
                        S[:, base + 1024 * u:base + 1024 * (u + 1)],
                        pj[:], CD12[:])
                    j3eng = nc.vector if sub % 2 == 0 else nc.gpsimd
                    j3eng.tensor_add(sl_j3, sl_j3, CD3[:])
                    nc.gpsimd.tensor_add(sl_up, sl_up, cup[:])
                    if u == 1:
                        nc.sync.dma_start(
                            out=slab[g][:, base:base + 4096],
                            in_=S[:, base:base + 4096])
                    ctx_stack.close()

    nc.compile()
    return nc


def _program() -> bass.Bass:
    global _PROGRAM
    if _PROGRAM is None:
        _PROGRAM = _build_program()
    return _PROGRAM


# --------------------------------------------------------------------------
# host entry point
# --------------------------------------------------------------------------

def _assemble(results):
    """8 per-core result dicts -> full [512, 512, 256] output."""
    out = np.empty((N, N, CH), np.float32)
    for k in range(NCORES):
        r = results[k]
        # halves h = sub//2: [g, p, h, {J12 2048, J3 1024, dg 512, up 512}]
        m = (np.asarray(r["slab"]).reshape(8, 128, 2, 4096)
             .astype(np.float32))
        slab = np.empty((RB, N, CH), np.float32)
        # main J blocks: [g, p, h, 1024*u + 512*Jh + 256*q + c]
        j12 = m[:, :, :, 0:2048].reshape(8, 128, 2, 2, 2, 2, CH)
        slab[:, 128:384, :] = (
            j12.transpose(0, 2, 3, 5, 4, 1, 6)    # g, h, u, q, Jh, p, c
            .reshape(RB, 2 * 128, CH))
        j3 = m[:, :, :, 2048:3072].reshape(8, 128, 2, 2, 2, CH)
        slab[:, 384:512, :] = (
            j3.transpose(0, 2, 3, 4, 1, 5).reshape(RB, 128, CH))
        # [diag|upper] per sub: [g, q*64+s, h, 3072 + 512*u + 256*du + c]
        du = m[:, :, :, 3072:4096].reshape(8, 2, 64, 2, 2, 2, CH)
        slab[:, 0:64, :] = (
            du[:, :, :, :, :, 0].transpose(0, 3, 4, 1, 2, 5)
            .reshape(RB, 64, CH))
        slab[:, 64:128, :] = (
            du[:, :, :, :, :, 1].transpose(0, 3, 4, 1, 2, 5)
            .reshape(RB, 64, CH))
        base = RB * k
        out[base:base + RB] = np.roll(slab, base, axis=1)
    return out


def build_in_maps(x, W, b):
    shared = _shared_inputs(W, b)
    return [dict(shared, **_core_inputs(x, k)) for k in range(NCORES)]


def kernel(x, W, b):
    nc = _program()
    in_maps = build_in_maps(x, W, b)
    res = run_bass_kernel_spmd(nc, in_maps, core_ids=list(range(NCORES)))
    return _assemble(res.results)

